# revision 4
# baseline (speedup 1.0000x reference)
"""2D Haar DWT (level 1) Trainium2 Bass kernel — fp16 I/O.

Input  x: [16, 64, 256, 256] f32
Output y: [16, 256, 128, 128] f32, y[n, s*64+c, i, j] = Haar mix s of the
2x2 block x[n, c, 2i:2i+2, 2j:2j+2].

Sharding: pure data parallel over the batch dim — core k gets batches
[2k, 2k+2).

The transform is pure data movement (out bytes == in bytes), so the kernel
is HBM-bound: per-core f32 traffic would be 67 MB (~187 us at the 358 GB/s
per-NC HBM limit). The rel-err budget (2e-2) admits fp16, halving traffic
to 33.5 MB/core (~94 us roofline; ~406 GB/s observed while busy -> ~85 us).
The host:
  - scales x by 0.5 (exact power of two — folds the whole Haar
    normalization, so the device does pure +/- butterflies),
  - casts to fp16,
  - de-interleaves even/odd columns to [n, c, h, 2, 128] so BOTH device
    butterfly stages are unit-stride (DVE 2x_1P perf mode needs 16-bit
    dtype + step 1 + 4B alignment; a stride-2 stage would run 1x),
  - un-scrambles the device's subband-pair-major output layout and upcasts
    fp16 -> f32 on the way out.

Per-core device pipeline, G=32 channels per group (4 groups of 4 MB):
  load  x[n, c0:c0+32]  -> it[p=(c,q), (o t j)]  one contiguous 4 MB DMA
                           (p = c*4+q holds rows [64q, 64q+64) of channel c)
  stage1 (vertical):     sdv[:,0/1] = rows 2r +/- 2r+1     (2 DVE ops, 2x)
  stage2 (horizontal):   oadd = t0+t1 (DVE); osub = t0-t1 split DVE/GpSimd
                         (v1 trace: DVE busy 78 us was the critical path at
                         99 us total; GpSimd TT (~58 G elem/s) absorbs 1/8
                         of the element work to pull DVE under the DMA time)
  store  oadd, osub      two contiguous 2 MB DMAs into a kernel-private
                         y layout [n, group, tile, p, f]
Loads ride the sync HWDGE ring, stores the scalar ring, so loads never
queue behind stores.
"""

import sys

sys.path.insert(0, "/opt/trn_rl_repo")

import numpy as np

import concourse.bacc as bacc
import concourse.mybir as mybir
from concourse.tile import TileContext

N_CORES = 8
N_PER_CORE = 2  # batches per core
C = 64  # input channels
H = 256
W = 256
W2 = W // 2
G = 32  # channels per group (4 MB loads)
NG = C // G  # groups per batch item
Q = 128 // G  # partitions per channel
F16 = mybir.dt.float16
WSPLIT = 32  # osub w-index where DVE hands off to GpSimd (of 2*G=64)


def build_nc():
    nc = bacc.Bacc("TRN2", target_bir_lowering=False, debug=False)
    x = nc.dram_tensor("x", [N_PER_CORE, C, H, 2, W2], F16, kind="ExternalInput")
    # Kernel-private output layout: [n, group, tile(oadd/osub), p, f].
    # The host unscrambles this to [n, 4C, H/2, W2] during the f32 upcast.
    y = nc.dram_tensor(
        "y", [N_PER_CORE, NG, 2, 128, G * 256], F16, kind="ExternalOutput"
    )

    with TileContext(nc) as tc:
        with (
            tc.tile_pool(name="inpool", bufs=2) as inpool,
            tc.tile_pool(name="sdpool", bufs=2) as sdpool,
            tc.tile_pool(name="outpool", bufs=2) as outpool,
        ):
            for n in range(N_PER_CORE):
                for g in range(NG):
                    c0 = g * G
                    # --- load: pure reshape of the 4 MB contiguous group.
                    # it[p, (o t j)] = x[n, c0 + p//Q, 64*(p%Q) + o, t, j]
                    it = inpool.tile([128, G * 512], F16, tag="in")
                    src = x[n, c0 : c0 + G].rearrange(
                        "c (q o) t j -> (c q) (o t j)", q=Q
                    )
                    nc.sync.dma_start(out=it[:], in_=src)

                    # --- stage 1 (vertical): rows 2r / 2r+1 within a partition
                    itv = it[:].rearrange("p (r u f) -> p r u f", r=G, u=2)
                    sd = sdpool.tile([128, G * 512], F16, tag="sd")
                    sdv = sd[:].rearrange("p (v r f) -> p v r f", v=2, r=G)
                    nc.vector.tensor_add(
                        out=sdv[:, 0], in0=itv[:, :, 0], in1=itv[:, :, 1]
                    )
                    nc.vector.tensor_sub(
                        out=sdv[:, 1], in0=itv[:, :, 0], in1=itv[:, :, 1]
                    )

                    # --- stage 2 (horizontal): even/odd column planes (both
                    # unit-stride thanks to the host de-interleave)
                    sdt = sd[:].rearrange("p (w t j) -> p w t j", t=2, j=W2)
                    oadd = outpool.tile([128, G * 256], F16, tag="oadd")
                    osub = outpool.tile([128, G * 256], F16, tag="osub")
                    oav = oadd[:].rearrange("p (w j) -> p w j", j=W2)
                    osv = osub[:].rearrange("p (w j) -> p w j", j=W2)
                    nc.vector.tensor_add(out=oav, in0=sdt[:, :, 0], in1=sdt[:, :, 1])
                    nc.vector.tensor_sub(
                        out=osv[:, :WSPLIT],
                        in0=sdt[:, :WSPLIT, 0],
                        in1=sdt[:, :WSPLIT, 1],
                    )
                    nc.gpsimd.tensor_sub(
                        out=osv[:, WSPLIT:],
                        in0=sdt[:, WSPLIT:, 0],
                        in1=sdt[:, WSPLIT:, 1],
                    )

                    # --- stores: two fully-contiguous 2 MB DMAs
                    nc.scalar.dma_start(out=y[n, g, 0], in_=oadd[:])
                    nc.scalar.dma_start(out=y[n, g, 1], in_=osub[:])

    nc.finalize()
    return nc


_NC = None


def _get_nc():
    global _NC
    if _NC is None:
        _NC = build_nc()
    return _NC


def _make_in_maps(x: np.ndarray) -> list[dict]:
    """Host prep: *0.5, cast fp16, de-interleave even/odd columns."""
    x = np.asarray(x)
    assert x.shape == (16, C, H, W), x.shape
    xr = x.reshape(16, C, H, W2, 2)
    xp = np.empty((16, C, H, 2, W2), dtype=np.float16)
    np.multiply(xr[..., 0], np.float32(0.5), out=xp[:, :, :, 0, :])
    np.multiply(xr[..., 1], np.float32(0.5), out=xp[:, :, :, 1, :])
    return [
        {"x": xp[k * N_PER_CORE : (k + 1) * N_PER_CORE]} for k in range(N_CORES)
    ]


def _gather(results: list[dict]) -> np.ndarray:
    y16 = np.concatenate([r["y"] for r in results], axis=0)  # [16,NG,2,128,G*256]
    # Device layout -> [n, s*C + c, i, j]:
    #   y16[n, g, e, (c q), (v r j)] ; s = 2e+v, c_full = g*G + c, i = q*G + r
    y16 = y16.reshape(16, NG, 2, G, Q, 2, G, W2)
    #                  n   g  e  c  q  v  r  j  -> n (e v) (g c) (q r) j
    y16 = y16.transpose(0, 2, 5, 1, 3, 4, 6, 7)
    return np.ascontiguousarray(y16).astype(np.float32).reshape(16, 4 * C, H // 2, W2)


def kernel(x: np.ndarray) -> np.ndarray:
    from concourse.bass_utils import run_bass_kernel_spmd

    nc = _get_nc()
    in_maps = _make_in_maps(x)
    res = run_bass_kernel_spmd(nc, in_maps, core_ids=list(range(N_CORES)))
    return _gather(res.results)


# revision 5
# speedup vs baseline: 1.1344x; 1.1344x over previous
"""2D Haar DWT (level 1) Trainium2 Bass kernel — fp16 I/O.

Input  x: [16, 64, 256, 256] f32
Output y: [16, 256, 128, 128] f32, y[n, s*64+c, i, j] = Haar mix s of the
2x2 block x[n, c, 2i:2i+2, 2j:2j+2].

Sharding: pure data parallel over the batch dim — core k gets batches
[2k, 2k+2).

The transform is pure data movement (out bytes == in bytes), so the kernel
is HBM-bound: per-core f32 traffic would be 67 MB (~187 us at the 358 GB/s
per-NC HBM limit). The rel-err budget (2e-2) admits fp16, halving traffic
to 33.5 MB/core (~94 us roofline; ~406 GB/s observed while busy -> ~85 us).
The host:
  - scales x by 0.5 (exact power of two — folds the whole Haar
    normalization, so the device does pure +/- butterflies),
  - casts to fp16,
  - de-interleaves even/odd columns to [n, c, h, 2, 128] so BOTH device
    butterfly stages are unit-stride (DVE 2x_1P perf mode needs 16-bit
    dtype + step 1 + 4B alignment; a stride-2 stage would run 1x),
  - un-scrambles the device's subband-pair-major output layout and upcasts
    fp16 -> f32 on the way out.

Per-core device pipeline, G=16 channels per group (8 groups of 2 MB):
  load  x[n, c0:c0+16]  -> it[p=(c,q), (o t j)]  one contiguous 2 MB DMA
                           (p = c*8+q holds rows [32q, 32q+32) of channel c)
  stage1 (vertical):     sdv[:,0/1] = rows 2r +/- 2r+1     (2 DVE ops, 2x)
  stage2 (horizontal):   oadd/osub  = t=0 +/- t=1 planes   (2 DVE ops, 2x)
  store  oadd, osub      two contiguous 1 MB DMAs into a kernel-private
                         y layout [n, group, tile, p, f]
All butterflies stay on DVE: a GpSimd tensor_tensor offload was measured
NEGATIVE (DVE TT 69.6us -> 91.8us from SBUF contention while GpSimd
streamed, net 99us -> 122us). Loads ride the sync HWDGE ring, stores the
scalar ring, so loads never queue behind stores; deep inpool buffering
(bufs=4) keeps the load queue full so DMA never idles waiting on compute.
"""

import sys

sys.path.insert(0, "/opt/trn_rl_repo")

import numpy as np

import concourse.bacc as bacc
import concourse.mybir as mybir
from concourse.tile import TileContext

N_CORES = 8
N_PER_CORE = 2  # batches per core
C = 64  # input channels
H = 256
W = 256
W2 = W // 2
G = 16  # channels per group (2 MB loads)
NG = C // G  # groups per batch item
Q = 128 // G  # partitions per channel
F16 = mybir.dt.float16


def build_nc():
    nc = bacc.Bacc("TRN2", target_bir_lowering=False, debug=False)
    x = nc.dram_tensor("x", [N_PER_CORE, C, H, 2, W2], F16, kind="ExternalInput")
    # Kernel-private output layout: [n, group, tile(oadd/osub), p, f].
    # The host unscrambles this to [n, 4C, H/2, W2] during the f32 upcast.
    y = nc.dram_tensor(
        "y", [N_PER_CORE, NG, 2, 128, G * 256], F16, kind="ExternalOutput"
    )

    with TileContext(nc) as tc:
        with (
            tc.tile_pool(name="inpool", bufs=4) as inpool,
            tc.tile_pool(name="sdpool", bufs=2) as sdpool,
            tc.tile_pool(name="outpool", bufs=3) as outpool,
        ):
            for n in range(N_PER_CORE):
                for g in range(NG):
                    c0 = g * G
                    # --- load: pure reshape of the 2 MB contiguous group.
                    # it[p, (o t j)] = x[n, c0 + p//Q, (H//Q)*(p%Q) + o, t, j]
                    it = inpool.tile([128, G * 512], F16, tag="in")
                    src = x[n, c0 : c0 + G].rearrange(
                        "c (q o) t j -> (c q) (o t j)", q=Q
                    )
                    nc.sync.dma_start(out=it[:], in_=src)

                    # --- stage 1 (vertical): rows 2r / 2r+1 within a partition
                    itv = it[:].rearrange("p (r u f) -> p r u f", r=G, u=2)
                    sd = sdpool.tile([128, G * 512], F16, tag="sd")
                    sdv = sd[:].rearrange("p (v r f) -> p v r f", v=2, r=G)
                    nc.vector.tensor_add(
                        out=sdv[:, 0], in0=itv[:, :, 0], in1=itv[:, :, 1]
                    )
                    nc.vector.tensor_sub(
                        out=sdv[:, 1], in0=itv[:, :, 0], in1=itv[:, :, 1]
                    )

                    # --- stage 2 (horizontal): even/odd column planes (both
                    # unit-stride thanks to the host de-interleave)
                    sdt = sd[:].rearrange("p (w t j) -> p w t j", t=2, j=W2)
                    oadd = outpool.tile([128, G * 256], F16, tag="oadd")
                    osub = outpool.tile([128, G * 256], F16, tag="osub")
                    oav = oadd[:].rearrange("p (w j) -> p w j", j=W2)
                    osv = osub[:].rearrange("p (w j) -> p w j", j=W2)
                    nc.vector.tensor_add(out=oav, in0=sdt[:, :, 0], in1=sdt[:, :, 1])
                    nc.vector.tensor_sub(out=osv, in0=sdt[:, :, 0], in1=sdt[:, :, 1])

                    # --- stores: two fully-contiguous 1 MB DMAs
                    nc.scalar.dma_start(out=y[n, g, 0], in_=oadd[:])
                    nc.scalar.dma_start(out=y[n, g, 1], in_=osub[:])

    nc.finalize()
    return nc


_NC = None


def _get_nc():
    global _NC
    if _NC is None:
        _NC = build_nc()
    return _NC


def _make_in_maps(x: np.ndarray) -> list[dict]:
    """Host prep: *0.5, cast fp16, de-interleave even/odd columns."""
    x = np.asarray(x)
    assert x.shape == (16, C, H, W), x.shape
    xr = x.reshape(16, C, H, W2, 2)
    xp = np.empty((16, C, H, 2, W2), dtype=np.float16)
    np.multiply(xr[..., 0], np.float32(0.5), out=xp[:, :, :, 0, :])
    np.multiply(xr[..., 1], np.float32(0.5), out=xp[:, :, :, 1, :])
    return [
        {"x": xp[k * N_PER_CORE : (k + 1) * N_PER_CORE]} for k in range(N_CORES)
    ]


def _gather(results: list[dict]) -> np.ndarray:
    y16 = np.concatenate([r["y"] for r in results], axis=0)  # [16,NG,2,128,G*256]
    # Device layout -> [n, s*C + c, i, j]:
    #   y16[n, g, e, (c q), (v r j)] ; s = 2e+v, c_full = g*G + c, i = q*G + r
    y16 = y16.reshape(16, NG, 2, G, Q, 2, G, W2)
    #                  n   g  e  c  q  v  r  j  -> n (e v) (g c) (q r) j
    y16 = y16.transpose(0, 2, 5, 1, 3, 4, 6, 7)
    return np.ascontiguousarray(y16).astype(np.float32).reshape(16, 4 * C, H // 2, W2)


def kernel(x: np.ndarray) -> np.ndarray:
    from concourse.bass_utils import run_bass_kernel_spmd

    nc = _get_nc()
    in_maps = _make_in_maps(x)
    res = run_bass_kernel_spmd(nc, in_maps, core_ids=list(range(N_CORES)))
    return _gather(res.results)
